# revision 5
# baseline (speedup 1.0000x reference)
"""Soft decision-tree layer (depth 4, 16 leaves) on 8 trn2 NeuronCores.

Sharding: 4-way data parallel (1024-token groups) x 2-way expert parallel
(8 leaves per core, one level-3 subtree half).  Each core computes, for
its 1024 tokens t and its 8 leaves l:
  partial[t,:] = sum_l path_l(t) * (x[t] @ Wl[l]) + sum_l path_l(t)*bl[l]
Host upcasts + sums the 2 expert partials per token group.

Everything is SBUF-resident (xt 2MB + wl 16MB fp16 + acc 4MB fp32), no
token groups.  GEMM operands are fp16 (1 col/cycle, 216ns per
128x128x512 matmul measured); fp32 accumulation in PSUM/SBUF.

Design notes (from trace analysis of the previous 253us version):
- decisions/path/bias run in the input-DMA shadow before wl0 lands
- the path-weighted bias sum is a K=32 PE matmul (pathT.T @ bl) instead
  of 8 DVE tensor-ops per tile (saves ~80us of DVE time)
- pathT comes from PE transpose-mode (DVE lanes can't cross partitions)
- host pre-swizzles all inputs so every DMA is contiguous per partition
- warmup matmuls use a DMA'd tile (no DVE memset dependency: the DVE
  engine preamble would delay the first matmul by ~3us)
- output DMA'd as fp16 (halves tail DMA; host upcasts)
- filler matmuls keep the PE HAM clock-gate warm across the wl0 wait
"""

import numpy as np

B, S, H = 2, 2048, 1024
DP, EP = 4, 2            # data-parallel x expert-parallel = 8 cores
T = (B * S) // DP        # 1024 tokens per core
LPC = 16 // EP           # 8 leaves per core
NT = T // 128            # 8 token tiles per core
KC = H // 128            # 8 contraction chunks
ND = 24                  # decision columns (22 used + 2 pad)

_prog_cache = {}


def _build_program():
    if "nc" in _prog_cache:
        return _prog_cache["nc"]

    from contextlib import ExitStack
    import concourse.bacc as bacc
    import concourse.tile as tile
    import concourse.mybir as mybir

    f32 = mybir.dt.float32
    f16 = mybir.dt.float16
    MULT = mybir.AluOpType.mult
    ADD = mybir.AluOpType.add
    SIG = mybir.ActivationFunctionType.Sigmoid

    nc = bacc.Bacc("TRN2", target_bir_lowering=False, debug=False, num_devices=8)

    xt_d = nc.dram_tensor("xt", [128, KC * T], f16, kind="ExternalInput").ap()
    wl_d = nc.dram_tensor("wl", [LPC, 128, KC * H], f16, kind="ExternalInput").ap()
    wd_d = nc.dram_tensor("wd", [128, KC * ND], f16, kind="ExternalInput").ap()
    bd_d = nc.dram_tensor("bd", [1, ND], f16, kind="ExternalInput").ap()
    bl_d = nc.dram_tensor("bl", [32, H], f16, kind="ExternalInput").ap()
    ones_d = nc.dram_tensor("ones", [1, 128], f16, kind="ExternalInput").ap()
    warm_d = nc.dram_tensor("warm", [128, 512], f16, kind="ExternalInput").ap()
    iden_d = nc.dram_tensor("iden", [128, 128], f32, kind="ExternalInput").ap()
    out_d = nc.dram_tensor("out", [T, H], f16, kind="ExternalOutput").ap()

    with tile.TileContext(nc) as tc, ExitStack() as ctx:
        consts = ctx.enter_context(tc.tile_pool(name="consts", bufs=1))
        xt_pool = ctx.enter_context(tc.tile_pool(name="xt", bufs=1))
        wl_pool = ctx.enter_context(tc.tile_pool(name="wl", bufs=1))
        acc_pool = ctx.enter_context(tc.tile_pool(name="acc", bufs=1))
        dec_pool = ctx.enter_context(tc.tile_pool(name="dec", bufs=2))
        out_pool = ctx.enter_context(tc.tile_pool(name="o16", bufs=4))
        ps_pool = ctx.enter_context(tc.tile_pool(name="ps", bufs=8, space="PSUM"))

        # --- tiny DMAs first (warm tile + consts), then the bulk stream ---
        warm = consts.tile([128, 512], f16, tag="warm")
        nc.sync.dma_start(warm[:], warm_d[:, :])
        ones = consts.tile([1, 128], f16, tag="ones")
        nc.sync.dma_start(ones[:], ones_d[:, :])
        wd_sb = consts.tile([128, KC * ND], f16, tag="wd")
        nc.sync.dma_start(wd_sb[:], wd_d[:, :])
        bd_sb = consts.tile([1, ND], f16, tag="bd")
        nc.sync.dma_start(bd_sb[:], bd_d[:, :])
        bl_sb = consts.tile([32, H], f16, tag="bl")
        nc.sync.dma_start(bl_sb[:], bl_d[:, :])
        iden = consts.tile([128, 128], f32, tag="iden")
        nc.sync.dma_start(iden[:], iden_d[:, :])

        xt = xt_pool.tile([128, KC * T], f16, tag="xt")
        nc.sync.dma_start(xt[:], xt_d[:, :])
        wls = []
        for l in range(LPC):
            w = wl_pool.tile([128, KC * H], f16, tag=f"wl{l}", name=f"wl{l}")
            nc.sync.dma_start(w[:], wl_d[l])
            wls.append(w)

        def xk(k, t):  # stationary: chunk k, token tile t
            return xt[:, k * T + t * 128:k * T + (t + 1) * 128]

        # --- PE warmup on the DMA'd tile (HAM ramp + fills DMA wait) ---
        wps = ps_pool.tile([128, 512], f32, tag="ps", name="warmps")
        for i in range(16):
            nc.tensor.matmul(wps[:], warm[:, 0:128], warm[:],
                             start=True, stop=True)

        # bd broadcast to 128 partitions via ones-vector matmul
        bdb = consts.tile([128, ND], f32, tag="bdb")
        bp = ps_pool.tile([128, 512], f32, tag="ps", name="bdps")
        nc.tensor.matmul(bp[:, 0:ND], ones[:], bd_sb[:], start=True, stop=True)
        nc.vector.tensor_copy(bdb[:], bp[:, 0:ND])

        # --- decisions: 7+1 psum chains over all 8 token tiles ---
        dec_sb = dec_pool.tile([128, NT * ND], f32, tag="dec", bufs=1)
        path = dec_pool.tile([128, NT * 32], f32, tag="path", bufs=1)
        nc.vector.memset(path[:], 0.0)
        pathT = dec_pool.tile([32, NT * 128], f16, tag="pathT", bufs=1)
        accs = [acc_pool.tile([128, H], f32, tag=f"acc{t}", name=f"acc{t}")
                for t in range(NT)]

        def sig_path(t, dps):
            # sigmoid(dec + bd) then the 8 path columns for this tile
            tadd = dec_pool.tile([128, ND], f32, tag="tadd", name=f"tadd{t}")
            nc.vector.tensor_tensor(tadd[:], dps, bdb[:], op=ADD)
            dsl = dec_sb[:, t * ND:(t + 1) * ND]
            nc.scalar.activation(dsl, tadd[:], SIG)
            # leaf local l = c2*4 + c1*2 + c0
            # cols: 0,1 root(c0); 2+2*c0+c1; 6+2*(2c1+c0)+c2; 14+l
            d1 = dsl[:, 2:6].rearrange("p (i c) -> p i c", c=2)
            d2 = dsl[:, 6:14].rearrange("p (j c) -> p j c", c=2)
            p4 = dec_pool.tile([128, 4], f32, tag="p4", name=f"p4_{t}")
            # p4[2c1+c0] = root[c0] * d1[c0, c1]
            nc.vector.tensor_tensor(p4[:, 0:2], dsl[:, 0:2], d1[:, :, 0], op=MULT)
            nc.vector.tensor_tensor(p4[:, 2:4], dsl[:, 0:2], d1[:, :, 1], op=MULT)
            p8 = dec_pool.tile([128, 8], f32, tag="p8", name=f"p8_{t}")
            # p8[4c2+j] = p4[j] * d2[j, c2]
            nc.vector.tensor_tensor(p8[:, 0:4], p4[:], d2[:, :, 0], op=MULT)
            nc.vector.tensor_tensor(p8[:, 4:8], p4[:], d2[:, :, 1], op=MULT)
            pt = path[:, t * 32:t * 32 + 8]
            nc.vector.tensor_tensor(pt, p8[:], dsl[:, 14:22], op=MULT)

        def transpose_bias(t):
            # pathT tile block via PE transpose, then bias = pathT.T @ bl
            tp = ps_pool.tile([128, 512], f32, tag="ps", name=f"tp{t}")
            nc.tensor.transpose(tp[0:32, 0:128], path[:, t * 32:(t + 1) * 32],
                                iden[:])
            ptT = pathT[:, t * 128:(t + 1) * 128]
            nc.vector.tensor_copy(ptT, tp[0:32, 0:128])
            for h in range(2):
                bps = ps_pool.tile([128, 512], f32, tag="ps",
                                   name=f"bias{t}_{h}")
                nc.tensor.matmul(bps[:], ptT, bl_sb[:, h * 512:(h + 1) * 512],
                                 start=True, stop=True)
                nc.vector.tensor_copy(accs[t][:, h * 512:(h + 1) * 512], bps[:])

        dpss = {}
        for t in range(7):
            dpss[t] = ps_pool.tile([128, 512], f32, tag="ps", name=f"dp{t}")
        for k in range(KC):
            for t in range(7):
                nc.tensor.matmul(dpss[t][:, 0:ND], xk(k, t),
                                 wd_sb[:, k * ND:(k + 1) * ND],
                                 start=(k == 0), stop=(k == KC - 1))
        for t in range(7):
            sig_path(t, dpss[t][:, 0:ND])
        # tile 7's chain (after tile 0's buf freed)
        dps7 = ps_pool.tile([128, 512], f32, tag="ps", name="dp7")
        for k in range(KC):
            nc.tensor.matmul(dps7[:, 0:ND], xk(k, 7),
                             wd_sb[:, k * ND:(k + 1) * ND],
                             start=(k == 0), stop=(k == KC - 1))
        sig_path(7, dps7[:, 0:ND])

        for t in range(NT):
            transpose_bias(t)

        # fillers: keep HAM warm across the wl0 DMA wait (fresh psum tile:
        # wps's ring slot has been recycled by the decision chains by now)
        wps2 = ps_pool.tile([128, 512], f32, tag="ps", name="warmps2")
        for i in range(10):
            nc.tensor.matmul(wps2[:], warm[:, 0:128], warm[:],
                             start=True, stop=True)

        # --- leaf passes: l outer, t inner, k inner; evict with path col ---
        def evict(t, l, ps_t, half, out=None):
            pcol = path[:, t * 32 + l:t * 32 + l + 1]
            o = half * 512
            dst = accs[t][:, o:o + 512] if out is None else out[:, o:o + 512]
            nc.vector.scalar_tensor_tensor(
                dst, ps_t[:], pcol, accs[t][:, o:o + 512], op0=MULT, op1=ADD)

        for l in range(LPC):
            wl = wls[l]
            last = l == LPC - 1
            for t in range(NT):
                o16 = None
                if last:
                    o16 = out_pool.tile([128, H], f16, tag="o16",
                                        name=f"o16_{t}")
                psl = ps_pool.tile([128, 512], f32, tag="ps",
                                   name=f"pl{l}_{t}")
                psr = ps_pool.tile([128, 512], f32, tag="ps",
                                   name=f"pr{l}_{t}")
                for k in range(KC):
                    lhsT = xk(k, t)
                    nc.tensor.matmul(psl[:], lhsT,
                                     wl[:, k * H:k * H + 512],
                                     start=(k == 0), stop=(k == KC - 1))
                    nc.tensor.matmul(psr[:], lhsT,
                                     wl[:, k * H + 512:(k + 1) * H],
                                     start=(k == 0), stop=(k == KC - 1))
                evict(t, l, psl, 0, out=o16)
                if last:
                    nc.sync.dma_start(out_d[t * 128:(t + 1) * 128, 0:512],
                                      o16[:, 0:512])
                evict(t, l, psr, 1, out=o16)
                if last:
                    nc.sync.dma_start(out_d[t * 128:(t + 1) * 128, 512:1024],
                                      o16[:, 512:1024])

    nc.compile()
    _prog_cache["nc"] = nc
    return nc


def _swizzle_kp(a):
    """[K*128, F] -> [128, K*F] fp16, partition-major contiguous."""
    k, f = a.shape[0] // 128, a.shape[1]
    return np.ascontiguousarray(
        a.reshape(k, 128, f).transpose(1, 0, 2).reshape(128, k * f)
    ).astype(np.float16)


def _core_inputs(x, Wd, bd, Wl, bl):
    """Build the 8 per-core input dicts (host-side sharding)."""
    x2 = np.ascontiguousarray(x, dtype=np.float32).reshape(B * S, H)
    Wd = np.asarray(Wd, dtype=np.float32)
    bd = np.asarray(bd, dtype=np.float32)
    Wl = np.ascontiguousarray(Wl, dtype=np.float32)
    bl = np.asarray(bl, dtype=np.float32)

    xts = [_swizzle_kp(np.ascontiguousarray(x2[d * T:(d + 1) * T].T))
           for d in range(DP)]

    # per-subtree decision matrix [H, ND] and bias [ND]
    wd_cs, bd_cs = [], []
    for e in range(EP):
        wd_c = np.zeros((H, ND), dtype=np.float32)
        bd_c = np.zeros((1, ND), dtype=np.float32)
        wd_c[:, 0:2] = Wd[0]                    # root, both choices
        bd_c[0, 0:2] = bd[0]
        for i in range(2):                      # level-1 nodes 1,2
            wd_c[:, 2 + 2 * i:4 + 2 * i] = Wd[1 + i]
            bd_c[0, 2 + 2 * i:4 + 2 * i] = bd[1 + i]
        for j in range(4):                      # level-2 nodes 3..6
            wd_c[:, 6 + 2 * j:8 + 2 * j] = Wd[3 + j]
            bd_c[0, 6 + 2 * j:8 + 2 * j] = bd[3 + j]
        for m in range(8):                      # level-3 nodes 7..14, choice e
            wd_c[:, 14 + m] = Wd[7 + m, :, e]
            bd_c[0, 14 + m] = bd[7 + m, e]
        wd_cs.append(_swizzle_kp(wd_c))
        bd_cs.append(bd_c.astype(np.float16))

    wl_cs, bl_cs = [], []
    for e in range(EP):
        w8 = np.stack([_swizzle_kp(Wl[LPC * e + l]) for l in range(LPC)])
        wl_cs.append(np.ascontiguousarray(w8))
        blc = np.zeros((32, H), dtype=np.float32)
        blc[0:LPC] = bl[LPC * e:LPC * (e + 1)]
        bl_cs.append(blc.astype(np.float16))

    ones = np.ones((1, 128), dtype=np.float16)
    warm = np.zeros((128, 512), dtype=np.float16)
    iden = np.eye(128, dtype=np.float32)

    in_maps = []
    for c in range(8):
        d, e = c // EP, c % EP
        in_maps.append({
            "xt": xts[d],
            "wl": wl_cs[e],
            "wd": wd_cs[e],
            "bd": bd_cs[e],
            "bl": bl_cs[e],
            "ones": ones,
            "warm": warm,
            "iden": iden,
        })
    return in_maps


def kernel(x, Wd, bd, Wl, bl, _want_results=False):
    from concourse import bass_utils

    nc = _build_program()
    in_maps = _core_inputs(x, Wd, bd, Wl, bl)
    res = bass_utils.run_bass_kernel_spmd(nc, in_maps, list(range(8)))

    out = np.empty((DP, T, H), dtype=np.float32)
    for d in range(DP):
        out[d] = (res.results[d * EP]["out"].astype(np.float32)
                  + res.results[d * EP + 1]["out"].astype(np.float32))
    out = out.reshape(B, S, H)
    if _want_results:
        return out, res
    return out
